# revision 11
# baseline (speedup 1.0000x reference)
"""Trainium2 Bass kernel for nn_BitPosAttMD (sparse_attention).

Math (per batch b):
  x1n/x2n = BN(x1/x2)            -> folded host-side into Wb/Wc (tiny param op)
  fa = Wb' @ x1 + bb'            [64, N]   (N = H*W = 4096)
  fb = Wc' @ x2 + bc'            [64, N]
  f1 = Wd @ x + bd               [256, N]
  sT[n, m] = sum_k fb[k,n] fa[k,m]           (scores, transposed layout)
  p = softmax_n(s[m, :]) = e^{sT - c} / D[m],  D[m] = sum_n e^{sT[n,m] - c}
  A[m, n] = 1 - sigmoid(p)
  out[c, m] = alpha * sum_n f1[c,n] A[m,n] + x[c,m]

Kernel identity used: 1 - sigmoid(p) = 0.5 - 0.5*tanh(p/2), so
  out = alpha*(0.5*S1[c] - 0.5*(f1 @ T^T)) + x,  T = tanh(p/2), S1 = rowsum(f1)
(or a polynomial in p replacing tanh, knob below).

Sharding: 8 cores = 4 batches x 2 query-halves (m in [0,2048) / [2048,4096)).
Each core: full n = 4096, its own 2048 m-columns. SPMD program, per-core inputs.
"""

import sys
import os

for _p in ("/opt/trn_rl_repo",):
    if _p not in sys.path and os.path.isdir(_p):
        sys.path.insert(0, _p)

import numpy as np

import concourse.bass as bass
import concourse.mybir as mybir
import concourse.tile as tile
from concourse import bacc
from concourse.bass_utils import run_bass_kernel_spmd

F32 = mybir.dt.float32
F32R = mybir.dt.float32r
BF16 = mybir.dt.bfloat16
AF = mybir.ActivationFunctionType
OP = mybir.AluOpType

C = 256
CK = 64
NN = 4096          # full n (key) dim per batch
M = 2048           # per-core m (query) dim
SB = 512           # m superblock
NSB = M // SB
EPS = 1e-5
SHIFT = 25.0       # global score shift before exp (softmax-invariant, overflow-safe)

# sigmoid-path knob: "tanh" (exact, 2 ACT passes) | "quad" | "cubic" (DVE poly)
W_PATH = os.environ.get("BITPOS_W_PATH", "tanh")
# minimax fits of g(p)=1-sigmoid(p) on [0,1], highest-degree first
QUAD = (0.02746143, -0.25951026, 0.50049511)
CUBIC = (0.01564343, 0.00425689, -0.2509589, 0.50003285)


def build_nc():
    nc = bacc.Bacc("TRN2", target_bir_lowering=False)

    x1h = nc.dram_tensor("x1h", [C, M], F32R, kind="ExternalInput")
    x2f = nc.dram_tensor("x2f", [C, NN], F32R, kind="ExternalInput")
    xf = nc.dram_tensor("xf", [C, NN], F32R, kind="ExternalInput")
    xres = nc.dram_tensor("xres", [C, M], F32, kind="ExternalInput")
    wbt = nc.dram_tensor("wbt", [C, CK], F32R, kind="ExternalInput")
    wct = nc.dram_tensor("wct", [C, CK], F32R, kind="ExternalInput")
    bbe = nc.dram_tensor("bbe", [CK, 1], F32, kind="ExternalInput")
    bce = nc.dram_tensor("bce", [CK, 1], F32, kind="ExternalInput")
    wdt = nc.dram_tensor("wdt", [C, C], F32R, kind="ExternalInput")
    bdr = nc.dram_tensor("bdr", [1, C], F32, kind="ExternalInput")
    alf = nc.dram_tensor("alf", [128, 1], F32, kind="ExternalInput")
    out = nc.dram_tensor("out", [C, M], F32, kind="ExternalOutput")

    x1h_r = x1h.rearrange("(ko p) m -> p ko m", p=128)
    x2f_r = x2f.rearrange("(ko p) m -> p ko m", p=128)
    xf_r = xf.rearrange("(ko p) m -> p ko m", p=128)
    xres_r = xres.rearrange("(ko p) m -> p ko m", p=128)
    wbt_r = wbt.rearrange("(ko p) k -> p ko k", p=128)
    wct_r = wct.rearrange("(ko p) k -> p ko k", p=128)
    wdt_r = wdt.rearrange("(ko p) c -> p ko c", p=128)
    out_r = out.rearrange("(ko p) m -> p ko m", p=128)

    if W_PATH == "tanh":
        a0, cscale = 0.5, -0.5
    elif W_PATH == "quad":
        a0, cscale = QUAD[2], 1.0
    else:
        a0, cscale = CUBIC[3], 1.0

    with tile.TileContext(nc) as tc:
        with (
            tc.tile_pool(name="const", bufs=1) as const,
            tc.tile_pool(name="ld", bufs=3) as ld,
            tc.tile_pool(name="acts", bufs=1) as acts,
            tc.tile_pool(name="work", bufs=2) as work,
        ):
            # ---- params to SBUF ----
            wbt_sb = const.tile([128, 2, CK], F32R)
            nc.sync.dma_start(wbt_sb, wbt_r)
            wct_sb = const.tile([128, 2, CK], F32R)
            nc.sync.dma_start(wct_sb, wct_r)
            bbe_sb = const.tile([CK, 1], F32)
            nc.sync.dma_start(bbe_sb, bbe[:])
            bce_sb = const.tile([CK, 1], F32)
            nc.sync.dma_start(bce_sb, bce[:])
            wdt_sb = const.tile([128, 2, C], F32R)
            nc.sync.dma_start(wdt_sb, wdt_r)
            bdr_sb = const.tile([1, C], F32)
            nc.sync.dma_start(bdr_sb, bdr[:])
            alf_sb = const.tile([128, 1], F32)
            nc.sync.dma_start(alf_sb, alf[:])
            ones_row = const.tile([1, 128], F32)   # K=1 stationary (rank-1 adds)
            nc.vector.memset(ones_row, 1.0)
            ones_col = const.tile([128, 1], BF16)  # K=128, M=1 stationary (col sums)
            nc.vector.memset(ones_col, 1.0)
            nshift = const.tile([128, 1], F32)     # exp bias (score shift)
            nc.vector.memset(nshift, -SHIFT)

            fa_sb = acts.tile([CK, M], F32R)
            fb_sb = acts.tile([CK, NN], F32R)
            f1t_sb = acts.tile([128, NN // 128, C], BF16)

            # ---- fa/fb: 1x1 convs on x1/x2 (BN folded into weights) ----
            with tc.tile_pool(name="cpsum", bufs=2, space="PSUM") as cpsum:
                for src_r, w_sb, b_sb, dst, nm in (
                    (x1h_r, wbt_sb, bbe_sb, fa_sb, M),
                    (x2f_r, wct_sb, bce_sb, fb_sb, NN),
                ):
                    for ms in range(0, nm, SB):
                        xt = ld.tile([128, 2, SB], F32R, tag="xc")
                        nc.sync.dma_start(xt, src_r[:, :, ms:ms + SB])
                        ps = cpsum.tile([CK, SB], F32, tag="cps")
                        nc.tensor.matmul(ps, w_sb[:, 0], xt[:, 0], start=True, stop=False)
                        nc.tensor.matmul(ps, w_sb[:, 1], xt[:, 1], start=False, stop=True)
                        nc.scalar.activation(dst[:, ms:ms + SB], ps, AF.Identity, bias=b_sb)

            # ---- f1t = (Wd @ x + bd)^T as [n_part, n_blk, c] bf16; S1 = rowsum(f1) ----
            with (
                tc.tile_pool(name="f1psum", bufs=2, space="PSUM") as f1p,
                tc.tile_pool(name="s1psum", bufs=1, space="PSUM") as s1p,
            ):
                s1_ps = [s1p.tile([128, 1], F32, tag=f"s1_{h}", name=f"s1_ps{h}")
                         for h in range(2)]
                for j in range(NN // 128):
                    xt = ld.tile([128, 2, 128], F32R, tag="xnb")
                    nc.sync.dma_start(xt, xf_r[:, :, 128 * j:128 * j + 128])
                    ps = f1p.tile([128, C], F32, tag="f1ps")
                    nc.tensor.matmul(ps, xt[:, 0], wdt_sb[:, 0], start=True, stop=False)
                    nc.tensor.matmul(ps, xt[:, 1], wdt_sb[:, 1], start=False, stop=False)
                    nc.tensor.matmul(ps, ones_row, bdr_sb, start=False, stop=True)
                    nc.vector.tensor_copy(f1t_sb[:, j], ps)
                    for h in range(2):
                        nc.tensor.matmul(
                            s1_ps[h], f1t_sb[:, j, 128 * h:128 * h + 128], ones_col,
                            start=(j == 0), stop=(j == NN // 128 - 1),
                        )
                s1_sb = const.tile([128, 2], F32)
                for h in range(2):
                    nc.vector.tensor_copy(s1_sb[:, h:h + 1], s1_ps[h])

            # combine constants: out = ca*G + bias + xres
            ca_sb = const.tile([128, 1], F32)
            nc.vector.tensor_scalar(ca_sb, alf_sb, cscale, None, op0=OP.mult)
            bias_sb = const.tile([128, 2], F32)
            nc.vector.tensor_scalar(bias_sb, s1_sb, alf_sb, a0, op0=OP.mult, op1=OP.mult)

            # ---- main loop over m superblocks ----
            with (
                tc.tile_pool(name="spsum", bufs=2, space="PSUM") as sp,
                tc.tile_pool(name="gpsum", bufs=1, space="PSUM") as gp,
                tc.tile_pool(name="dpsum", bufs=1, space="PSUM") as dp,
                tc.tile_pool(name="upsum", bufs=1, space="PSUM") as up,
            ):
                for s in range(NSB):
                    ms = SB * s
                    q_sb = work.tile([128, NN // 128, SB], BF16, tag="q")

                    # A: scoresT (2 n-blocks per psum tile) + exp
                    for jj in range(NN // 256):
                        sps = sp.tile([128, 2, SB], F32, tag="sps")
                        for j2 in range(2):
                            j = 2 * jj + j2
                            nc.tensor.matmul(
                                sps[:, j2], fb_sb[:, 128 * j:128 * j + 128],
                                fa_sb[:, ms:ms + SB], start=True, stop=True,
                            )
                        nc.scalar.activation(
                            q_sb[:, 2 * jj:2 * jj + 2], sps, AF.Exp, bias=nshift,
                        )

                    # B: D[m] = sum_n q  (ones-matmul col sums, accumulate)
                    dps = dp.tile([1, SB], F32, tag="dps")
                    for j in range(NN // 128):
                        nc.tensor.matmul(
                            dps, ones_col, q_sb[:, j],
                            start=(j == 0), stop=(j == NN // 128 - 1),
                        )

                    # C: u = 1/D broadcast to [128, SB] bf16
                    d_sb = work.tile([1, SB], F32, tag="dsb")
                    nc.vector.tensor_copy(d_sb, dps)
                    invd = work.tile([1, SB], F32, tag="invd")
                    nc.vector.reciprocal(invd, d_sb)
                    ups = up.tile([128, SB], F32, tag="ups")
                    nc.tensor.matmul(ups, ones_row, invd, start=True, stop=True)
                    ubc = work.tile([128, SB], BF16, tag="ubc")
                    nc.vector.tensor_copy(ubc, ups)

                    # D: p = q*u ; W = sigmoid-part(p)   (in-place in q_sb)
                    for g in range(8):
                        sl = q_sb[:, 4 * g:4 * g + 4]
                        u_b = ubc[:, None, :].to_broadcast((128, 4, SB))
                        nc.vector.tensor_tensor(sl, sl, u_b, OP.mult)
                        if W_PATH == "tanh":
                            nc.scalar.activation(sl, sl, AF.Tanh, scale=0.5)
                        elif W_PATH == "quad":
                            wt = work.tile([128, 4, SB], BF16, tag="wt")
                            nc.vector.tensor_scalar(
                                wt, sl, QUAD[0], QUAD[1], op0=OP.mult, op1=OP.add)
                            nc.vector.tensor_tensor(sl, wt, sl, OP.mult)
                        else:  # cubic
                            wt = work.tile([128, 4, SB], BF16, tag="wt")
                            nc.vector.tensor_scalar(
                                wt, sl, CUBIC[0], CUBIC[1], op0=OP.mult, op1=OP.add)
                            nc.vector.tensor_tensor(wt, wt, sl, OP.mult)
                            nc.vector.tensor_scalar(wt, wt, CUBIC[2], None, op0=OP.add)
                            nc.vector.tensor_tensor(sl, wt, sl, OP.mult)

                    # E+F: G = f1 @ W^T ; out = ca*G + bias + xres
                    for h in range(2):
                        gps = gp.tile([128, SB], F32, tag=f"gps{h}")
                        for j in range(NN // 128):
                            nc.tensor.matmul(
                                gps, f1t_sb[:, j, 128 * h:128 * h + 128], q_sb[:, j],
                                start=(j == 0), stop=(j == NN // 128 - 1),
                            )
                        oc = work.tile([128, SB], F32, tag="oc")
                        nc.vector.tensor_scalar(
                            oc, gps, ca_sb, bias_sb[:, h:h + 1],
                            op0=OP.mult, op1=OP.add,
                        )
                        xrt = ld.tile([128, SB], F32, tag="xr")
                        nc.sync.dma_start(xrt, xres_r[:, h, ms:ms + SB])
                        nc.vector.tensor_tensor(oc, oc, xrt, OP.add)
                        nc.sync.dma_start(out_r[:, h, ms:ms + SB], oc)

    return nc


_NC_CACHE = None
LAST_RESULTS = None


def _get_nc():
    global _NC_CACHE
    if _NC_CACHE is None:
        nc = build_nc()
        if not nc.is_finalized():
            nc.finalize()
        _NC_CACHE = nc
    return _NC_CACHE


def kernel(x1, x2, x, bn_gamma, bn_beta, bn_mean, bn_var,
           Wb, bb, Wc, bc, Wd, bd, alpha):
    x1 = np.asarray(x1, np.float32)
    x2 = np.asarray(x2, np.float32)
    x = np.asarray(x, np.float32)
    B = x1.shape[0]
    N = x1.shape[2] * x1.shape[3]

    g = np.asarray(bn_gamma, np.float64)
    be = np.asarray(bn_beta, np.float64)
    mu = np.asarray(bn_mean, np.float64)
    var = np.asarray(bn_var, np.float64)
    scale = g / np.sqrt(var + EPS)
    shift = be - mu * scale
    Wb64 = np.asarray(Wb, np.float64)
    Wc64 = np.asarray(Wc, np.float64)
    wbt = np.ascontiguousarray((Wb64 * scale[None, :]).T.astype(np.float32))   # [C, CK]
    wct = np.ascontiguousarray((Wc64 * scale[None, :]).T.astype(np.float32))
    bbe = (Wb64 @ shift + np.asarray(bb, np.float64)).astype(np.float32)[:, None]
    bce = (Wc64 @ shift + np.asarray(bc, np.float64)).astype(np.float32)[:, None]
    wdt = np.ascontiguousarray(np.asarray(Wd, np.float32).T)                   # [C, C]
    bdr = np.ascontiguousarray(np.asarray(bd, np.float32)[None, :])            # [1, C]
    alf = np.full((128, 1), np.asarray(alpha, np.float32)[0], np.float32)

    in_maps = []
    for core in range(8):
        b, half = divmod(core, 2)
        m0 = half * M
        x1b = x1[b].reshape(C, N)
        xb = x[b].reshape(C, N)
        in_maps.append({
            "x1h": np.ascontiguousarray(x1b[:, m0:m0 + M]),
            "x2f": np.ascontiguousarray(x2[b].reshape(C, N)),
            "xf": np.ascontiguousarray(xb),
            "xres": np.ascontiguousarray(xb[:, m0:m0 + M]),
            "wbt": wbt, "wct": wct, "bbe": bbe, "bce": bce,
            "wdt": wdt, "bdr": bdr, "alf": alf,
        })

    trace = bool(int(os.environ.get("BITPOS_TRACE", "0")))
    res = None
    if trace:
        try:
            res = run_bass_kernel_spmd(
                _get_nc(), in_maps, list(range(8)), trace=True,
                tmpdir=os.environ.get("BITPOS_TMPDIR"))
        except Exception as e:  # fall back to an untraced run
            import traceback
            traceback.print_exc()
            print("trace run failed, retrying without trace:", e)
            res = None
    if res is None:
        res = run_bass_kernel_spmd(_get_nc(), in_maps, list(range(8)))
    global LAST_RESULTS
    LAST_RESULTS = res

    out = np.empty((B, C, N), np.float32)
    for core in range(8):
        b, half = divmod(core, 2)
        out[b][:, half * M:(half + 1) * M] = res.results[core]["out"]
    return out.reshape(x1.shape)


if __name__ == "__main__":
    import importlib.util

    spec = importlib.util.spec_from_file_location(
        "reference", os.path.join(os.path.dirname(__file__), "reference.py"))
    ref = importlib.util.module_from_spec(spec)
    spec.loader.exec_module(ref)
    inputs = {k: np.asarray(v) for k, v in ref.setup_inputs().items()}
    actual = kernel(**inputs)
    expected = np.asarray(ref.reference(**ref.setup_inputs()))
    rel = np.linalg.norm(actual - expected) / np.linalg.norm(expected)
    print("Relative error:", rel)


# revision 13
# speedup vs baseline: 1.1720x; 1.1720x over previous
"""Trainium2 Bass kernel for nn_BitPosAttMD (sparse_attention).

Math (per batch b, N = H*W = 4096):
  fa = Wb' @ x1 + bb'   [64, N]    (BN folded into Wb'/bb' host-side)
  fb = Wc' @ x2 + bc'   [64, N]
  f1 = Wd @ x + bd      [256, N]
  sT[n, m] = fb[:,n] . fa[:,m]            (scores, transposed layout)
  p[n, m] = e^{sT - c} / D[m],  D[m] = sum_n e^{sT[n,m] - c}   (softmax over n)
  A[m, n] = 1 - sigmoid(p) = 0.5 - 0.5*tanh(p/2)
  out[c, m] = alpha * sum_n f1[c,n] A[m,n] + x[c,m]
            = alpha*(0.5*S1[c] - 0.5*G[c,m]) + x,  G = f1 @ W^T, W = tanh(p/2)

W is tanh(p/2) on ScalarE for some n-block groups and a polynomial fit of
tanh(p/2) on VectorE for the rest (engine load balancing knob N_QUAD).

Sharding: 8 cores = 4 batches x 2 query-halves (m in [0,2048)/[2048,4096)).
Each core handles full n = 4096 and its own 2048 m-columns (SPMD program,
per-core input slices). The m loop is emitted software-pipelined (phase A of
superblock s+1 ahead of phases B..F of superblock s) to keep TensorE dense.
"""

import sys
import os

for _p in ("/opt/trn_rl_repo",):
    if _p not in sys.path and os.path.isdir(_p):
        sys.path.insert(0, _p)

import numpy as np

import concourse.bass as bass
import concourse.mybir as mybir
import concourse.tile as tile
from concourse import bacc
from concourse.bass_utils import run_bass_kernel_spmd

F32 = mybir.dt.float32
F32R = mybir.dt.float32r
BF16 = mybir.dt.bfloat16
AF = mybir.ActivationFunctionType
OP = mybir.AluOpType

C = 256
CK = 64
NN = 4096          # full n (key) dim per batch
M = 2048           # per-core m (query) dim
SB = 512           # m superblock
NSB = M // SB
NBLK = NN // 128   # 32 n-blocks
EPS = 1e-5
SHIFT = 25.0       # global score shift before exp (softmax-invariant)

# Of the 8 groups of 4 n-blocks per superblock, this many use the DVE
# polynomial for W (rest use ScalarE tanh). 0 = all tanh, 8 = all poly.
N_QUAD = int(os.environ.get("BITPOS_NQUAD", "4"))
# minimax fit of tanh(p/2) ~ B1*p + B2*p^2 on [0,1]  (max A err ~6e-4)
B1, B2 = 0.51475607, -0.05139474


def build_nc():
    nc = bacc.Bacc("TRN2", target_bir_lowering=False)

    x1h = nc.dram_tensor("x1h", [C, M], F32R, kind="ExternalInput")
    x2f = nc.dram_tensor("x2f", [C, NN], F32R, kind="ExternalInput")
    xf = nc.dram_tensor("xf", [C, NN], F32R, kind="ExternalInput")
    xres = nc.dram_tensor("xres", [C, M], F32, kind="ExternalInput")
    wbt2 = nc.dram_tensor("wbt2", [C, 128], F32R, kind="ExternalInput")
    wct2 = nc.dram_tensor("wct2", [C, 128], F32R, kind="ExternalInput")
    bbe2 = nc.dram_tensor("bbe2", [128, 1], F32, kind="ExternalInput")
    bce2 = nc.dram_tensor("bce2", [128, 1], F32, kind="ExternalInput")
    wdt = nc.dram_tensor("wdt", [C, C], F32R, kind="ExternalInput")
    bdr = nc.dram_tensor("bdr", [1, C], F32, kind="ExternalInput")
    alf = nc.dram_tensor("alf", [128, 1], F32, kind="ExternalInput")
    out = nc.dram_tensor("out", [C, M], F32, kind="ExternalOutput")

    x1h_r = x1h.rearrange("(ko p) m -> p ko m", p=128)
    x2f_r = x2f.rearrange("(ko p) m -> p ko m", p=128)
    xf_r = xf.rearrange("(ko p) m -> p ko m", p=128)
    xres_r = xres.rearrange("(ko p) m -> p ko m", p=128)
    wbt_r = wbt2.rearrange("(ko p) k -> p ko k", p=128)
    wct_r = wct2.rearrange("(ko p) k -> p ko k", p=128)
    wdt_r = wdt.rearrange("(ko p) c -> p ko c", p=128)
    out_r = out.rearrange("(ko p) m -> p ko m", p=128)

    with tile.TileContext(nc) as tc:
        with (
            tc.tile_pool(name="const", bufs=1) as const,
            tc.tile_pool(name="ld", bufs=3) as ld,
            tc.tile_pool(name="acts", bufs=1) as acts,
            tc.tile_pool(name="work", bufs=2) as work,
            tc.tile_pool(name="qpool", bufs=3) as qpool,
        ):
            # ---- params to SBUF ----
            wbt_sb = const.tile([128, 2, 128], F32R)
            nc.sync.dma_start(wbt_sb, wbt_r)
            wct_sb = const.tile([128, 2, 128], F32R)
            nc.sync.dma_start(wct_sb, wct_r)
            bbe_sb = const.tile([128, 1], F32)
            nc.sync.dma_start(bbe_sb, bbe2[:])
            bce_sb = const.tile([128, 1], F32)
            nc.sync.dma_start(bce_sb, bce2[:])
            wdt_sb = const.tile([128, 2, C], F32R)
            nc.sync.dma_start(wdt_sb, wdt_r)
            bdr_sb = const.tile([1, C], F32)
            nc.sync.dma_start(bdr_sb, bdr[:])
            alf_sb = const.tile([128, 1], F32)
            nc.sync.dma_start(alf_sb, alf[:])
            ones_row = const.tile([1, 128], F32)   # K=1 stationary (rank-1 adds)
            nc.vector.memset(ones_row, 1.0)
            ones_col = const.tile([128, 1], BF16)  # K=128, M=1 stationary (col sums)
            nc.vector.memset(ones_col, 1.0)
            nshift = const.tile([128, 1], F32)     # exp bias (score shift)
            nc.vector.memset(nshift, -SHIFT)

            fa_sb = acts.tile([128, M], BF16)      # fa duplicated on both halves
            fb_sb = acts.tile([128, NN], BF16)     # fb duplicated on both halves
            f1t_sb = acts.tile([128, NBLK, C], BF16)

            # ---- fa/fb: 1x1 convs on x1/x2 (weights duplicated -> M=128) ----
            with tc.tile_pool(name="cpsum", bufs=2, space="PSUM") as cpsum:
                for src_r, w_sb, b_sb, dst, nm in (
                    (x1h_r, wbt_sb, bbe_sb, fa_sb, M),
                    (x2f_r, wct_sb, bce_sb, fb_sb, NN),
                ):
                    for ms in range(0, nm, SB):
                        xt = ld.tile([128, 2, SB], F32R, tag="xc")
                        nc.sync.dma_start(xt, src_r[:, :, ms:ms + SB])
                        ps = cpsum.tile([128, SB], F32, tag="cps")
                        nc.tensor.matmul(ps, w_sb[:, 0], xt[:, 0], start=True, stop=False)
                        nc.tensor.matmul(ps, w_sb[:, 1], xt[:, 1], start=False, stop=True)
                        nc.scalar.activation(dst[:, ms:ms + SB], ps, AF.Identity, bias=b_sb)

            # ---- f1t = (Wd @ x + bd)^T as [n_part, n_blk, c] bf16; S1 = rowsum(f1) ----
            with (
                tc.tile_pool(name="f1psum", bufs=2, space="PSUM") as f1p,
                tc.tile_pool(name="s1psum", bufs=1, space="PSUM") as s1p,
            ):
                s1_ps = [s1p.tile([128, 1], F32, tag=f"s1_{h}", name=f"s1_ps{h}")
                         for h in range(2)]
                for j in range(NBLK):
                    xt = ld.tile([128, 2, 128], F32R, tag="xnb")
                    nc.sync.dma_start(xt, xf_r[:, :, 128 * j:128 * j + 128])
                    ps = f1p.tile([128, C], F32, tag="f1ps")
                    nc.tensor.matmul(ps, xt[:, 0], wdt_sb[:, 0], start=True, stop=False)
                    nc.tensor.matmul(ps, xt[:, 1], wdt_sb[:, 1], start=False, stop=False)
                    nc.tensor.matmul(ps, ones_row, bdr_sb, start=False, stop=True)
                    nc.vector.tensor_copy(f1t_sb[:, j], ps)
                    for h in range(2):
                        nc.tensor.matmul(
                            s1_ps[h], f1t_sb[:, j, 128 * h:128 * h + 128], ones_col,
                            start=(j == 0), stop=(j == NBLK - 1),
                        )
                s1_sb = const.tile([128, 2], F32)
                for h in range(2):
                    nc.vector.tensor_copy(s1_sb[:, h:h + 1], s1_ps[h])

            # combine constants: out = ca*G + bias + xres, ca = -alpha/2
            ca_sb = const.tile([128, 1], F32)
            nc.vector.tensor_scalar(ca_sb, alf_sb, -0.5, None, op0=OP.mult)
            bias_sb = const.tile([128, 2], F32)
            nc.vector.tensor_scalar(bias_sb, s1_sb, alf_sb, 0.5, op0=OP.mult, op1=OP.mult)

            # ---- main loop over m superblocks (software-pipelined emission) ----
            with (
                tc.tile_pool(name="spsum", bufs=2, space="PSUM") as sp,
                tc.tile_pool(name="gpsum", bufs=1, space="PSUM") as gp,
                tc.tile_pool(name="dpsum", bufs=1, space="PSUM") as dp,
                tc.tile_pool(name="upsum", bufs=1, space="PSUM") as up,
            ):
                q_tiles = {}

                def phase_a(s):
                    """scoresT (row-packed pairs) + exp -> q_tiles[s]"""
                    ms = SB * s
                    q_sb = qpool.tile([128, NBLK, SB], BF16, tag="q", name=f"q_{s}")
                    q_tiles[s] = q_sb
                    for jj in range(NBLK // 2):
                        sps = sp.tile([128, 2, SB], F32, tag="sps", name=f"sps_{s}_{jj}")
                        for j2 in range(2):
                            j = 2 * jj + j2
                            pb = 64 * j2
                            nc.tensor.matmul(
                                sps[:, j2],
                                fb_sb[pb:pb + 64, 128 * j:128 * j + 128],
                                fa_sb[pb:pb + 64, ms:ms + SB],
                                start=True, stop=True,
                                tile_position=(pb, 0),
                            )
                        nc.scalar.activation(
                            q_sb[:, 2 * jj:2 * jj + 2], sps, AF.Exp, bias=nshift,
                        )

                def phase_rest(s):
                    """denom, 1/D, broadcast, W, G, combine, store for superblock s"""
                    ms = SB * s
                    q_sb = q_tiles.pop(s)

                    # B: D[m] col-packed: group g sums all n for m-quarter g
                    dps = dp.tile([128, 128], F32, tag="dps", name=f"dps_{s}")
                    for j in range(NBLK):
                        for g in range(4):
                            nc.tensor.matmul(
                                dps[32 * g:32 * g + 1, :], ones_col,
                                q_sb[:, j, 128 * g:128 * g + 128],
                                start=(j == 0), stop=(j == NBLK - 1),
                                tile_position=(0, 32 * g),
                            )

                    # C: u = 1/D -> row layout -> broadcast [128, SB] bf16
                    # (full-tile reciprocal: rows other than 0/32/64/96 are
                    # don't-care; DVE partition step must be 1)
                    invd = work.tile([128, 128], F32, tag="invd", name=f"invd_{s}")
                    nc.vector.reciprocal(invd, dps)
                    invd_row = work.tile([1, SB], F32, tag="invdr", name=f"invdr_{s}")
                    for g in range(4):
                        nc.sync.dma_start(
                            invd_row[:, 128 * g:128 * g + 128],
                            invd[32 * g:32 * g + 1, :])
                    ups = up.tile([128, SB], F32, tag="ups", name=f"ups_{s}")
                    nc.tensor.matmul(ups, ones_row, invd_row, start=True, stop=True)
                    ubc = work.tile([128, SB], BF16, tag="ubc", name=f"ubc_{s}")
                    nc.vector.tensor_copy(ubc, ups)

                    # D: p = q*u ; W = tanh(p/2) (ACT) or B1*p+B2*p^2 (DVE)
                    for g in range(8):
                        sl = q_sb[:, 4 * g:4 * g + 4]
                        u_b = ubc[:, None, :].to_broadcast((128, 4, SB))
                        nc.vector.tensor_tensor(sl, sl, u_b, OP.mult)
                        if g < N_QUAD:
                            wt = work.tile([128, 4, SB], BF16, tag="wt",
                                           name=f"wt_{s}_{g}")
                            nc.vector.tensor_scalar(
                                wt, sl, B2, B1, op0=OP.mult, op1=OP.add)
                            nc.vector.tensor_tensor(sl, wt, sl, OP.mult)
                        else:
                            nc.scalar.activation(sl, sl, AF.Tanh, scale=0.5)

                    # E+F: G = f1 @ W^T ; out = ca*G + bias + xres
                    for h in range(2):
                        gps = gp.tile([128, SB], F32, tag=f"gps{h}", name=f"gps{h}_{s}")
                        for j in range(NBLK):
                            nc.tensor.matmul(
                                gps, f1t_sb[:, j, 128 * h:128 * h + 128], q_sb[:, j],
                                start=(j == 0), stop=(j == NBLK - 1),
                            )
                        oc = work.tile([128, SB], F32, tag="oc", name=f"oc_{s}_{h}")
                        nc.vector.tensor_scalar(
                            oc, gps, ca_sb, bias_sb[:, h:h + 1],
                            op0=OP.mult, op1=OP.add,
                        )
                        xrt = ld.tile([128, SB], F32, tag="xr", name=f"xr_{s}_{h}")
                        nc.sync.dma_start(xrt, xres_r[:, h, ms:ms + SB])
                        nc.vector.tensor_tensor(oc, oc, xrt, OP.add)
                        nc.sync.dma_start(out_r[:, h, ms:ms + SB], oc)

                for s in range(NSB):
                    phase_a(s)
                    if s >= 1:
                        phase_rest(s - 1)
                phase_rest(NSB - 1)

    return nc


_NC_CACHE = None
LAST_RESULTS = None


def _get_nc():
    global _NC_CACHE
    if _NC_CACHE is None:
        nc = build_nc()
        if not nc.is_finalized():
            nc.finalize()
        _NC_CACHE = nc
    return _NC_CACHE


def kernel(x1, x2, x, bn_gamma, bn_beta, bn_mean, bn_var,
           Wb, bb, Wc, bc, Wd, bd, alpha):
    x1 = np.asarray(x1, np.float32)
    x2 = np.asarray(x2, np.float32)
    x = np.asarray(x, np.float32)
    B = x1.shape[0]
    N = x1.shape[2] * x1.shape[3]

    g = np.asarray(bn_gamma, np.float64)
    be = np.asarray(bn_beta, np.float64)
    mu = np.asarray(bn_mean, np.float64)
    var = np.asarray(bn_var, np.float64)
    scale = g / np.sqrt(var + EPS)
    shift = be - mu * scale
    Wb64 = np.asarray(Wb, np.float64)
    Wc64 = np.asarray(Wc, np.float64)
    wbt = (Wb64 * scale[None, :]).T.astype(np.float32)      # [C, CK]
    wct = (Wc64 * scale[None, :]).T.astype(np.float32)
    bbe = (Wb64 @ shift + np.asarray(bb, np.float64)).astype(np.float32)[:, None]
    bce = (Wc64 @ shift + np.asarray(bc, np.float64)).astype(np.float32)[:, None]
    wbt2 = np.ascontiguousarray(np.concatenate([wbt, wbt], axis=1))   # [C, 128]
    wct2 = np.ascontiguousarray(np.concatenate([wct, wct], axis=1))
    bbe2 = np.ascontiguousarray(np.concatenate([bbe, bbe], axis=0))   # [128, 1]
    bce2 = np.ascontiguousarray(np.concatenate([bce, bce], axis=0))
    wdt = np.ascontiguousarray(np.asarray(Wd, np.float32).T)          # [C, C]
    bdr = np.ascontiguousarray(np.asarray(bd, np.float32)[None, :])   # [1, C]
    alf = np.full((128, 1), np.asarray(alpha, np.float32)[0], np.float32)

    in_maps = []
    for core in range(8):
        b, half = divmod(core, 2)
        m0 = half * M
        x1b = x1[b].reshape(C, N)
        xb = x[b].reshape(C, N)
        in_maps.append({
            "x1h": np.ascontiguousarray(x1b[:, m0:m0 + M]),
            "x2f": np.ascontiguousarray(x2[b].reshape(C, N)),
            "xf": np.ascontiguousarray(xb),
            "xres": np.ascontiguousarray(xb[:, m0:m0 + M]),
            "wbt2": wbt2, "wct2": wct2, "bbe2": bbe2, "bce2": bce2,
            "wdt": wdt, "bdr": bdr, "alf": alf,
        })

    trace = bool(int(os.environ.get("BITPOS_TRACE", "0")))
    res = None
    if trace:
        try:
            res = run_bass_kernel_spmd(
                _get_nc(), in_maps, list(range(8)), trace=True,
                tmpdir=os.environ.get("BITPOS_TMPDIR"))
        except Exception as e:  # fall back to an untraced run
            import traceback
            traceback.print_exc()
            print("trace run failed, retrying without trace:", e)
            res = None
    if res is None:
        res = run_bass_kernel_spmd(_get_nc(), in_maps, list(range(8)))
    global LAST_RESULTS
    LAST_RESULTS = res

    out = np.empty((B, C, N), np.float32)
    for core in range(8):
        b, half = divmod(core, 2)
        out[b][:, half * M:(half + 1) * M] = res.results[core]["out"]
    return out.reshape(x1.shape)


if __name__ == "__main__":
    import importlib.util

    spec = importlib.util.spec_from_file_location(
        "reference", os.path.join(os.path.dirname(__file__), "reference.py"))
    ref = importlib.util.module_from_spec(spec)
    spec.loader.exec_module(ref)
    inputs = {k: np.asarray(v) for k, v in ref.setup_inputs().items()}
    actual = kernel(**inputs)
    expected = np.asarray(ref.reference(**ref.setup_inputs()))
    rel = np.linalg.norm(actual - expected) / np.linalg.norm(expected)
    print("Relative error:", rel)
